# revision 14
# baseline (speedup 1.0000x reference)
"""Trainium2 Bass kernel for the MoE-routing attack-net problem.

Strategy:
  - Host: stable-sort samples by routed class, split each class across the
    8 cores, pad each (core, class) chunk to CAP=3584 samples -> each core
    processes S=35840 samples in 70 tiles of 512, 7 tiles per class.
  - Device (SPMD, identical program on 8 cores): encoder + the one routed
    decoder per tile. Matmuls run in fp16 with a hi/lo split (3 accumulating
    terms: Wh@ah + Wh@al + Wl@ah) which is fp32-grade accurate (~2e-7) but
    streams at 1 cycle/row instead of fp32's 4. All activations carry a
    fixed 2048x scale (Lrelu is positively homogeneous; the scale cancels
    in the final Lp-normalize) so fp16 lo-parts stay in normal range.
    Tiles are packed into the PE array via tile_position (2 tiles for the
    64-col layers, 4 for the 32-col layers) and run concurrently.
  - Drains: ACT does Lrelu+bias (fp32 out), DVE converts to fp16 hi,
    GPSIMD/DVE subtract for the fp16 lo. Lp-normalize tail on dense
    [128, 512] collector blocks.
  - Host: invert the permutation, return [262144, 2] float32.
"""

import os
import sys
from contextlib import ExitStack

import numpy as np

sys.path.insert(0, "/opt/trn_rl_repo")

import concourse.bacc as bacc  # noqa: E402
import concourse.bass as bass  # noqa: E402
import concourse.mybir as mybir  # noqa: E402
import concourse.tile as tile  # noqa: E402

N = 262144
C = 10
NCORES = 8
CAP = 3584              # per-class per-core capacity (multiple of 512)
S = C * CAP             # samples per core (padded)
F = 512                 # tile free size
T = S // F              # 70 tiles per core
PAIRS = T // 2          # 35
QUADS = (T + 3) // 4    # 18 (last quad only has 2 tiles)
BLOCKS = (QUADS + 15) // 16  # 2
EPS = 0.5
P_NORM_SCALE = 2.0 ** 0.5 * EPS   # d^(1/p) * eps
SLOPE = 0.01
SC = 2048.0             # global activation scale (cancels in the normalize)

# fp16 weight blob column layout (hi and lo parts)
WH_WE1 = 0              # [6, 64]  K-stacked [Wh;Wh;Wl], rows 0-5 and 64-69
WH_WE2 = 64             # hi [50, 128] rows 0-49 and 64-113
WL_WE2 = 192            # lo
WH_WD1 = 320            # 10 x hi [100, 64] rows 0-99
WL_WD1 = 960
WH_WD2 = 1600           # 10 x hi [50, 32] rows 0-49 and 64-113
WL_WD2 = 1920
WH_WD3 = 2240           # 10 x hi [15, 32] rows 0-14, 32-46, 64-78, 96-110
WL_WD3 = 2560
WH_COLS = 2880

# fp32 blob (tail matmuls)
W32_SSEL = 0            # [128, 64] pair-sum (scaled by 1/P_NORM_SCALE^2)
W32_RREP = 64           # [64, 128] replicate
W32_COLS = 192

BB_BE1 = 0
BB_BE2 = 1
BB_BD1 = 2              # 35 per-pair columns
BB_BD2 = 2 + PAIRS      # 18 per-quad columns
BB_COLS = 2 + PAIRS + QUADS

F32 = mybir.dt.float32
F16 = mybir.dt.float16

_CACHE = {}


def _tile_class(t):
    return t // (CAP // F)


def build_program():
    nc = bacc.Bacc("TRN2", target_bir_lowering=False, debug=False)
    xt = nc.dram_tensor("xt", [6, S], F16, kind="ExternalInput").ap()
    wh = nc.dram_tensor("wh", [128, WH_COLS], F16, kind="ExternalInput").ap()
    w32 = nc.dram_tensor("w32", [128, W32_COLS], F32, kind="ExternalInput").ap()
    bb = nc.dram_tensor("bb", [128, BB_COLS], F32, kind="ExternalInput").ap()
    tb = nc.dram_tensor("tb", [128, BLOCKS], F32, kind="ExternalInput").ap()
    out = nc.dram_tensor("out", [128, BLOCKS * F], F32, kind="ExternalOutput").ap()

    LR = mybir.ActivationFunctionType.Lrelu
    ADD = mybir.AluOpType.add
    MULT = mybir.AluOpType.mult
    SUB = mybir.AluOpType.subtract

    with tile.TileContext(nc) as tc, ExitStack() as ctx:
        persist = ctx.enter_context(tc.tile_pool(name="persist", bufs=1))
        sb = ctx.enter_context(tc.tile_pool(name="sb", bufs=3))
        sbq = ctx.enter_context(tc.tile_pool(name="sbq", bufs=3))
        ps1 = ctx.enter_context(tc.tile_pool(name="ps1", bufs=1, space="PSUM"))
        ps2 = ctx.enter_context(tc.tile_pool(name="ps2", bufs=3, space="PSUM"))
        ps3 = ctx.enter_context(tc.tile_pool(name="ps3", bufs=2, space="PSUM"))
        psl = ctx.enter_context(tc.tile_pool(name="psl", bufs=2, space="PSUM"))

        wht = persist.tile([128, WH_COLS], F16)
        nc.sync.dma_start(out=wht, in_=wh)
        w32t = persist.tile([128, W32_COLS], F32)
        nc.sync.dma_start(out=w32t, in_=w32)
        bbt = persist.tile([128, BB_COLS], F32)
        nc.sync.dma_start(out=bbt, in_=bb)
        tbt = persist.tile([128, BLOCKS], F32)
        nc.sync.dma_start(out=tbt, in_=tb)
        coll = persist.tile([128, BLOCKS * F], F32)
        nc.gpsimd.memset(coll, 1.0)

        def mm3(pt, orow, ow, lrow, k, hcol, lcol, rh, rl):
            # three accumulating fp16 terms: Wh@ah + Wh@al + Wl@ah
            nc.tensor.matmul(pt[orow:orow + ow],
                             wht[lrow:lrow + k, hcol:hcol + ow], rh,
                             start=True, stop=False,
                             tile_position=(lrow, orow))
            nc.tensor.matmul(pt[orow:orow + ow],
                             wht[lrow:lrow + k, hcol:hcol + ow], rl,
                             start=False, stop=False,
                             tile_position=(lrow, orow))
            nc.tensor.matmul(pt[orow:orow + ow],
                             wht[lrow:lrow + k, lcol:lcol + ow], rh,
                             start=False, stop=True,
                             tile_position=(lrow, orow))

        def drain(pt_ap, bias_col, a32_t, hi_t, lo_t, conv_on_act=False):
            # Lrelu+bias (fp32), fp16 hi convert, fp16 lo residual
            nc.scalar.activation(out=a32_t, in_=pt_ap, func=LR,
                                 bias=bbt[:, bias_col:bias_col + 1], alpha=SLOPE)
            if conv_on_act:
                nc.scalar.activation(out=hi_t, in_=a32_t,
                                     func=mybir.ActivationFunctionType.Copy,
                                     bias=0.0)
            else:
                nc.vector.tensor_scalar(hi_t, a32_t, 1.0, None, MULT)
            nc.vector.tensor_tensor(lo_t, a32_t, hi_t, SUB)

        a1_pair = [None, None]   # (a1h, a1l) per pair in current quad

        for p in range(PAIRS):
            tA, tB = 2 * p, 2 * p + 1
            cA, cB = _tile_class(tA), _tile_class(tB)

            # x (pre-split hi/lo, K-stacked [xh;xl;xh]): A rows 0-5, B rows 64-69
            xtt = sb.tile([70, F], F16, tag="xtt")
            nc.sync.dma_start(out=xtt[0:6, :], in_=xt[:, tA * F:(tA + 1) * F])
            nc.sync.dma_start(out=xtt[64:70, :], in_=xt[:, tB * F:(tB + 1) * F])

            # L1: [2]->[50] (padded 64); single K-stacked MM per tile
            p1 = ps1.tile([128, F], F32)
            nc.tensor.matmul(p1[0:64, :], wht[0:6, WH_WE1:WH_WE1 + 64],
                             xtt[0:6, :], start=True, stop=True,
                             tile_position=(0, 0))
            nc.tensor.matmul(p1[64:128, :], wht[64:70, WH_WE1:WH_WE1 + 64],
                             xtt[64:70, :], start=True, stop=True,
                             tile_position=(64, 64))
            h1_32 = sb.tile([128, F], F32, tag="h1_32")
            h1h = sb.tile([128, F], F16, tag="h1h")
            h1l = sb.tile([128, F], F16, tag="h1l")
            drain(p1, BB_BE1, h1_32, h1h, h1l, conv_on_act=True)

            # L2: [50]->[100] (padded 128); A rows 0-49, B rows 64-113
            p2a = ps2.tile([128, F], F32, tag="ps2")
            p2b = ps2.tile([128, F], F32, tag="ps2")
            mm3(p2a, 0, 128, 0, 50, WH_WE2, WL_WE2,
                h1h[0:50, :], h1l[0:50, :])
            mm3(p2b, 0, 128, 64, 50, WH_WE2, WL_WE2,
                h1h[64:114, :], h1l[64:114, :])
            hh = sb.tile([128, 2, F], F16, tag="hh")
            hl = sb.tile([128, 2, F], F16, tag="hl")
            hA32 = sb.tile([128, F], F32, tag="hA32")
            nc.scalar.activation(out=hA32, in_=p2a, func=LR,
                                 bias=bbt[:, BB_BE2:BB_BE2 + 1], alpha=SLOPE)
            nc.vector.tensor_scalar(hh[:, 0, :], hA32, 1.0, None, MULT)
            nc.vector.tensor_tensor(hl[:, 0, :], hA32, hh[:, 0, :], SUB)
            hB32 = sb.tile([128, F], F32, tag="hB32")
            nc.scalar.activation(out=hB32, in_=p2b, func=LR,
                                 bias=bbt[:, BB_BE2:BB_BE2 + 1], alpha=SLOPE)
            nc.vector.tensor_scalar(hh[:, 1, :], hB32, 1.0, None, MULT)
            nc.vector.tensor_tensor(hl[:, 1, :], hB32, hh[:, 1, :], SUB)

            # L3: [100]->[50] (padded 64); A at c0, B at c64
            p3 = ps3.tile([128, F], F32)
            mm3(p3, 0, 64, 0, 100, WH_WD1 + cA * 64, WL_WD1 + cA * 64,
                hh[0:100, 0, :], hl[0:100, 0, :])
            mm3(p3, 64, 64, 0, 100, WH_WD1 + cB * 64, WL_WD1 + cB * 64,
                hh[0:100, 1, :], hl[0:100, 1, :])
            a1_32 = sb.tile([128, F], F32, tag="a1_32")
            a1h = sb.tile([128, F], F16, tag="a1h")
            a1l = sb.tile([128, F], F16, tag="a1l")
            drain(p3, BB_BD1 + p, a1_32, a1h, a1l, conv_on_act=True)
            a1_pair[p % 2] = (a1h, a1l)

            if p % 2 == 1 or p == PAIRS - 1:
                # quad stage over tiles 4q .. 4q+3 (last quad: 2 tiles)
                q = p // 2
                full = p % 2 == 1
                a1ah, a1al = a1_pair[0]
                if full:
                    a1bh, a1bl = a1_pair[1]
                tq = 4 * q
                cs = [_tile_class(min(tq + i, T - 1)) for i in range(4)]

                # L4: [50]->[15] (padded 32); 4 tiles -> cols 0/32/64/96
                p4 = psl.tile([128, F], F32, tag="psl")
                mm3(p4, 0, 32, 0, 50, WH_WD2 + cs[0] * 32, WL_WD2 + cs[0] * 32,
                    a1ah[0:50, :], a1al[0:50, :])
                mm3(p4, 32, 32, 64, 50, WH_WD2 + cs[1] * 32, WL_WD2 + cs[1] * 32,
                    a1ah[64:114, :], a1al[64:114, :])
                if full:
                    mm3(p4, 64, 32, 0, 50, WH_WD2 + cs[2] * 32, WL_WD2 + cs[2] * 32,
                        a1bh[0:50, :], a1bl[0:50, :])
                    mm3(p4, 96, 32, 64, 50, WH_WD2 + cs[3] * 32, WL_WD2 + cs[3] * 32,
                        a1bh[64:114, :], a1bl[64:114, :])
                else:
                    nc.vector.memset(p4[64:128, :], 0.0)
                a2_32 = sbq.tile([128, F], F32, tag="a2_32")
                a2h = sbq.tile([128, F], F16, tag="a2h")
                a2l = sbq.tile([128, F], F16, tag="a2l")
                drain(p4, BB_BD2 + q, a2_32, a2h, a2l)

                # L5: [15]->[2] (padded 32); diag placement
                p5 = psl.tile([128, F], F32, tag="psl")
                mm3(p5, 0, 32, 0, 15, WH_WD3 + cs[0] * 32, WL_WD3 + cs[0] * 32,
                    a2h[0:15, :], a2l[0:15, :])
                mm3(p5, 32, 32, 32, 15, WH_WD3 + cs[1] * 32, WL_WD3 + cs[1] * 32,
                    a2h[32:47, :], a2l[32:47, :])
                if full:
                    mm3(p5, 64, 32, 64, 15, WH_WD3 + cs[2] * 32, WL_WD3 + cs[2] * 32,
                        a2h[64:79, :], a2l[64:79, :])
                    mm3(p5, 96, 32, 96, 15, WH_WD3 + cs[3] * 32, WL_WD3 + cs[3] * 32,
                        a2h[96:111, :], a2l[96:111, :])
                else:
                    nc.vector.memset(p5[64:128, :], 1.0)
                stage = sbq.tile([128, F], F32, tag="stage")
                nc.scalar.activation(
                    out=stage, in_=p5,
                    func=mybir.ActivationFunctionType.Copy, bias=0.0)

                # gather the 8 useful rows into the collector block
                b = q // 16
                r0 = (q % 16) * 8
                for i in range(4):
                    nc.sync.dma_start(
                        out=coll[r0 + 2 * i:r0 + 2 * i + 2, b * F:(b + 1) * F],
                        in_=stage[32 * i:32 * i + 2, :])

            if (p % 2 == 1 and (p // 2) % 16 == 15) or p == PAIRS - 1:
                # tail for completed block (values carry the SC scale, which
                # cancels in o/|o|; only the bias blob is pre-scaled)
                b = (p // 2) // 16
                colv = coll[:, b * F:(b + 1) * F]
                ob = sbq.tile([128, F], F32, tag="ob")
                nc.vector.tensor_scalar(ob, colv, tbt[:, b:b + 1], None, ADD)
                sq = sbq.tile([128, F], F32, tag="sq")
                nc.vector.tensor_tensor(sq, ob, ob, MULT)
                p6 = psl.tile([64, F], F32, tag="psl")
                nc.tensor.matmul(p6[0:64, :], w32t[0:128, W32_SSEL:W32_SSEL + 64],
                                 sq[0:128, :], start=True, stop=True)
                nrm = sbq.tile([64, F], F32, tag="nrm")
                nc.scalar.activation(out=nrm, in_=p6[0:64, :],
                                     func=mybir.ActivationFunctionType.Sqrt)
                rcp = sbq.tile([64, F], F32, tag="rcp")
                nc.vector.reciprocal(rcp, nrm)
                p7 = psl.tile([128, F], F32, tag="psl")
                nc.tensor.matmul(p7, w32t[0:64, W32_RREP:W32_RREP + 128],
                                 rcp[0:64, :], start=True, stop=True)
                ov = sbq.tile([128, F], F32, tag="ov")
                nc.vector.tensor_tensor(ov, ob, p7, MULT)
                oc = sbq.tile([128, F], F32, tag="oc")
                nc.vector.tensor_scalar(oc, ov, -EPS, EPS,
                                        mybir.AluOpType.max, mybir.AluOpType.min)
                nc.sync.dma_start(out=out[:, b * F:(b + 1) * F], in_=oc)

    nc.compile()
    return nc


def _split16(a):
    hi = a.astype(np.float16)
    lo = (a - hi.astype(np.float32)).astype(np.float16)
    return hi, lo


def _host_prep(x, y, We1, be1, We2, be2, Wd1, bd1, Wd2, bd2, Wd3, bd3):
    x = np.ascontiguousarray(np.asarray(x, dtype=np.float32))
    y = np.asarray(y).astype(np.int64)
    order = np.argsort(y, kind="stable")
    counts = np.bincount(y, minlength=C)

    # per-(class, core) contiguous shares; x scaled by SC and hi/lo split
    xs = x * np.float32(SC)
    xh = xs.astype(np.float16)
    xl = (xs - xh.astype(np.float32)).astype(np.float16)
    xt_cores = np.zeros((NCORES, 6, S), dtype=np.float16)
    seg_meta = []
    pos = 0
    for c in range(C):
        cnt = int(counts[c])
        base, rem = divmod(cnt, NCORES)
        off = 0
        for k in range(NCORES):
            n_k = base + (1 if k < rem else 0)
            assert n_k <= CAP, f"class {c} share {n_k} exceeds CAP {CAP}"
            idx = order[pos + off: pos + off + n_k]
            sl = slice(c * CAP, c * CAP + n_k)
            xt_cores[k, 0:2, sl] = xh[idx].T
            xt_cores[k, 2:4, sl] = xl[idx].T
            xt_cores[k, 4:6, sl] = xh[idx].T
            seg_meta.append((k, c, n_k, idx))
            off += n_k
        pos += cnt

    We1 = np.asarray(We1, np.float32); be1 = np.asarray(be1, np.float32)
    We2 = np.asarray(We2, np.float32); be2 = np.asarray(be2, np.float32)
    Wd1 = np.asarray(Wd1, np.float32); bd1 = np.asarray(bd1, np.float32)
    Wd2 = np.asarray(Wd2, np.float32); bd2 = np.asarray(bd2, np.float32)
    Wd3 = np.asarray(Wd3, np.float32); bd3 = np.asarray(bd3, np.float32)

    WH = np.zeros((128, WH_COLS), dtype=np.float16)
    e1h, e1l = _split16(We1)
    for r0 in (0, 64):
        WH[r0 + 0:r0 + 2, WH_WE1:WH_WE1 + 50] = e1h
        WH[r0 + 2:r0 + 4, WH_WE1:WH_WE1 + 50] = e1h
        WH[r0 + 4:r0 + 6, WH_WE1:WH_WE1 + 50] = e1l
    e2h, e2l = _split16(We2)
    for r0 in (0, 64):
        WH[r0:r0 + 50, WH_WE2:WH_WE2 + 100] = e2h
        WH[r0:r0 + 50, WL_WE2:WL_WE2 + 100] = e2l
    for c in range(C):
        d1h, d1l = _split16(Wd1[c])
        WH[0:100, WH_WD1 + c * 64:WH_WD1 + c * 64 + 50] = d1h
        WH[0:100, WL_WD1 + c * 64:WL_WD1 + c * 64 + 50] = d1l
        d2h, d2l = _split16(Wd2[c])
        for r0 in (0, 64):
            WH[r0:r0 + 50, WH_WD2 + c * 32:WH_WD2 + c * 32 + 15] = d2h
            WH[r0:r0 + 50, WL_WD2 + c * 32:WL_WD2 + c * 32 + 15] = d2l
        d3h, d3l = _split16(Wd3[c])
        for r0 in (0, 32, 64, 96):
            WH[r0:r0 + 15, WH_WD3 + c * 32:WH_WD3 + c * 32 + 2] = d3h
            WH[r0:r0 + 15, WL_WD3 + c * 32:WL_WD3 + c * 32 + 2] = d3l

    W32 = np.zeros((128, W32_COLS), dtype=np.float32)
    rr = np.arange(128)
    W32[rr, W32_SSEL + rr // 2] = 1.0 / (P_NORM_SCALE * P_NORM_SCALE)
    jj = np.arange(64)
    W32[jj, W32_RREP + 2 * jj] = 1.0
    W32[jj, W32_RREP + 2 * jj + 1] = 1.0

    BB = np.zeros((128, BB_COLS), dtype=np.float32)
    BB[0:50, BB_BE1] = be1 * SC
    BB[64:114, BB_BE1] = be1 * SC
    BB[0:100, BB_BE2] = be2 * SC
    for p in range(PAIRS):
        cA, cB = _tile_class(2 * p), _tile_class(2 * p + 1)
        BB[0:50, BB_BD1 + p] = bd1[cA] * SC
        BB[64:114, BB_BD1 + p] = bd1[cB] * SC
    for q in range(QUADS):
        cs = [_tile_class(min(4 * q + i, T - 1)) for i in range(4)]
        BB[0:15, BB_BD2 + q] = bd2[cs[0]] * SC
        BB[32:47, BB_BD2 + q] = bd2[cs[1]] * SC
        BB[64:79, BB_BD2 + q] = bd2[cs[2]] * SC
        BB[96:111, BB_BD2 + q] = bd2[cs[3]] * SC

    TB = np.zeros((128, BLOCKS), dtype=np.float32)
    for t in range(T):
        q = t // 4
        b, row0 = q // 16, (q % 16) * 8 + (t % 4) * 2
        cls = _tile_class(t)
        TB[row0, b] = bd3[cls, 0] * SC
        TB[row0 + 1, b] = bd3[cls, 1] * SC

    return xt_cores, WH, W32, BB, TB, seg_meta


def _unpack(results, seg_meta):
    final = np.empty((N, 2), dtype=np.float32)
    per_core = []
    for k in range(NCORES):
        OUT = results[k]["out"]
        o = np.empty((S, 2), dtype=np.float32)
        for t in range(T):
            q, r = t // 4, t % 4
            b, row0 = q // 16, (q % 16) * 8 + r * 2
            o[t * F:(t + 1) * F, 0] = OUT[row0, b * F:(b + 1) * F]
            o[t * F:(t + 1) * F, 1] = OUT[row0 + 1, b * F:(b + 1) * F]
        per_core.append(o)
    for (k, c, n_k, idx) in seg_meta:
        final[idx] = per_core[k][c * CAP:c * CAP + n_k]
    return final


def kernel(**inputs):
    if "nc" not in _CACHE:
        _CACHE["nc"] = build_program()
    nc = _CACHE["nc"]

    xt_cores, WH, W32, BB, TB, seg_meta = _host_prep(**inputs)

    if os.environ.get("BASS_KERNEL_SIM"):
        from concourse.bass_interp import CoreSim
        results = []
        for k in range(int(os.environ.get("BASS_KERNEL_SIM_CORES", "1"))):
            sim = CoreSim(nc, trace=False, require_finite=False,
                          require_nnan=False)
            sim.tensor("xt")[:] = xt_cores[k]
            sim.tensor("wh")[:] = WH
            sim.tensor("w32")[:] = W32
            sim.tensor("bb")[:] = BB
            sim.tensor("tb")[:] = TB
            sim.simulate(check_with_hw=False)
            results.append({"out": np.array(sim.tensor("out"))})
        while len(results) < NCORES:
            results.append(results[-1])
    else:
        from concourse.bass_utils import run_bass_kernel_spmd
        in_maps = [{"xt": xt_cores[k], "wh": WH, "w32": W32, "bb": BB,
                    "tb": TB} for k in range(NCORES)]
        res = run_bass_kernel_spmd(nc, in_maps, core_ids=list(range(NCORES)),
                                   **_CACHE.get("run_kwargs", {}))
        _CACHE["last_result"] = res
        results = res.results

    return _unpack(results, seg_meta)


# revision 15
# speedup vs baseline: 1.1978x; 1.1978x over previous
"""Trainium2 Bass kernel for the MoE-routing attack-net problem.

Strategy:
  - Host: stable-sort samples by routed class, split each class across the
    8 cores, pad each (core, class) chunk to CAP=3584 samples -> each core
    processes S=35840 samples in 70 tiles of 512, 7 tiles per class.
  - Device (SPMD, identical program on 8 cores): encoder + the one routed
    decoder per tile. Matmuls run in fp16 with a hi/lo split (3 accumulating
    terms: Wh@ah + Wh@al + Wl@ah) which is fp32-grade accurate (~2e-7) but
    streams at 1 cycle/row instead of fp32's 4. All activations carry a
    fixed 2048x scale (Lrelu is positively homogeneous; the scale cancels
    in the final Lp-normalize) so fp16 lo-parts stay in normal range.
    Tiles are packed into the PE array via tile_position (2 tiles for the
    64-col layers, 4 for the 32-col layers) and run concurrently.
  - Drains: ACT does Lrelu+bias (fp32 out), DVE converts to fp16 hi,
    GPSIMD/DVE subtract for the fp16 lo. Lp-normalize tail on dense
    [128, 512] collector blocks.
  - Host: invert the permutation, return [262144, 2] float32.
"""

import os
import sys
from contextlib import ExitStack

import numpy as np

sys.path.insert(0, "/opt/trn_rl_repo")

import concourse.bacc as bacc  # noqa: E402
import concourse.bass as bass  # noqa: E402
import concourse.mybir as mybir  # noqa: E402
import concourse.tile as tile  # noqa: E402

N = 262144
C = 10
NCORES = 8
CAP = 3584              # per-class per-core capacity (multiple of 512)
S = C * CAP             # samples per core (padded)
F = 512                 # tile free size
T = S // F              # 70 tiles per core
PAIRS = T // 2          # 35
QUADS = (T + 3) // 4    # 18 (last quad only has 2 tiles)
BLOCKS = (QUADS + 15) // 16  # 2
EPS = 0.5
P_NORM_SCALE = 2.0 ** 0.5 * EPS   # d^(1/p) * eps
SLOPE = 0.01
SC = 2048.0             # global activation scale (cancels in the normalize)

# fp16 weight blob column layout (hi and lo parts)
WH_WE1 = 0              # [6, 64]  K-stacked [Wh;Wh;Wl], rows 0-5 and 64-69
WH_WE2 = 64             # hi [50, 128] rows 0-49 and 64-113
WL_WE2 = 192            # lo
WH_WD1 = 320            # 10 x hi [100, 64] rows 0-99
WL_WD1 = 960
WH_WD2 = 1600           # 10 x hi [50, 32] rows 0-49 and 64-113
WL_WD2 = 1920
WH_WD3 = 2240           # 10 x hi [15, 32] rows 0-14, 32-46, 64-78, 96-110
WL_WD3 = 2560
WH_COLS = 2880

# fp32 blob (tail matmuls)
W32_SSEL = 0            # [128, 64] pair-sum (scaled by 1/P_NORM_SCALE^2)
W32_RREP = 64           # [64, 128] replicate
W32_COLS = 192

BB_BE1 = 0
BB_BE2 = 1
BB_BD1 = 2              # 35 per-pair columns
BB_BD2 = 2 + PAIRS      # 18 per-quad columns
BB_COLS = 2 + PAIRS + QUADS

F32 = mybir.dt.float32
F16 = mybir.dt.float16

_CACHE = {}


def _tile_class(t):
    return t // (CAP // F)


def build_program():
    nc = bacc.Bacc("TRN2", target_bir_lowering=False, debug=False)
    xt = nc.dram_tensor("xt", [6, S], F16, kind="ExternalInput").ap()
    wh = nc.dram_tensor("wh", [128, WH_COLS], F16, kind="ExternalInput").ap()
    w32 = nc.dram_tensor("w32", [128, W32_COLS], F32, kind="ExternalInput").ap()
    bb = nc.dram_tensor("bb", [128, BB_COLS], F32, kind="ExternalInput").ap()
    tb = nc.dram_tensor("tb", [128, BLOCKS], F32, kind="ExternalInput").ap()
    out = nc.dram_tensor("out", [128, BLOCKS * F], F32, kind="ExternalOutput").ap()

    LR = mybir.ActivationFunctionType.Lrelu
    ADD = mybir.AluOpType.add
    MULT = mybir.AluOpType.mult
    SUB = mybir.AluOpType.subtract

    with tile.TileContext(nc) as tc, ExitStack() as ctx:
        persist = ctx.enter_context(tc.tile_pool(name="persist", bufs=1))
        sb = ctx.enter_context(tc.tile_pool(name="sb", bufs=3))
        sbq = ctx.enter_context(tc.tile_pool(name="sbq", bufs=3))
        ps1 = ctx.enter_context(tc.tile_pool(name="ps1", bufs=1, space="PSUM"))
        ps2 = ctx.enter_context(tc.tile_pool(name="ps2", bufs=3, space="PSUM"))
        ps3 = ctx.enter_context(tc.tile_pool(name="ps3", bufs=2, space="PSUM"))
        psl = ctx.enter_context(tc.tile_pool(name="psl", bufs=2, space="PSUM"))

        wht = persist.tile([128, WH_COLS], F16)
        nc.sync.dma_start(out=wht, in_=wh)
        w32t = persist.tile([128, W32_COLS], F32)
        nc.sync.dma_start(out=w32t, in_=w32)
        bbt = persist.tile([128, BB_COLS], F32)
        nc.sync.dma_start(out=bbt, in_=bb)
        tbt = persist.tile([128, BLOCKS], F32)
        nc.sync.dma_start(out=tbt, in_=tb)
        coll = persist.tile([128, BLOCKS * F], F32)
        nc.gpsimd.memset(coll, 1.0)

        def mm3(pt, orow, ow, lrow, k, hcol, lcol, rh, rl):
            # three accumulating fp16 terms: Wh@ah + Wh@al + Wl@ah
            nc.tensor.matmul(pt[orow:orow + ow],
                             wht[lrow:lrow + k, hcol:hcol + ow], rh,
                             start=True, stop=False,
                             tile_position=(lrow, orow))
            nc.tensor.matmul(pt[orow:orow + ow],
                             wht[lrow:lrow + k, hcol:hcol + ow], rl,
                             start=False, stop=False,
                             tile_position=(lrow, orow))
            nc.tensor.matmul(pt[orow:orow + ow],
                             wht[lrow:lrow + k, lcol:lcol + ow], rh,
                             start=False, stop=True,
                             tile_position=(lrow, orow))

        def drain(pt_ap, bias_col, a32_t, hi_t, lo_t, conv_on_act=False):
            # Lrelu+bias (fp32), fp16 hi convert, fp16 lo residual
            nc.scalar.activation(out=a32_t, in_=pt_ap, func=LR,
                                 bias=bbt[:, bias_col:bias_col + 1], alpha=SLOPE)
            if conv_on_act:
                nc.scalar.activation(out=hi_t, in_=a32_t,
                                     func=mybir.ActivationFunctionType.Copy,
                                     bias=0.0)
            else:
                nc.vector.tensor_scalar(hi_t, a32_t, 1.0, None, MULT)
            nc.vector.tensor_tensor(lo_t, a32_t, hi_t, SUB)

        a1_pair = [None, None]   # (a1h, a1l) per pair in current quad

        for p in range(PAIRS):
            tA, tB = 2 * p, 2 * p + 1
            cA, cB = _tile_class(tA), _tile_class(tB)

            # x (pre-split hi/lo, K-stacked [xh;xl;xh]): A rows 0-5, B rows 64-69
            xtt = sb.tile([70, F], F16, tag="xtt")
            nc.sync.dma_start(out=xtt[0:6, :], in_=xt[:, tA * F:(tA + 1) * F])
            nc.sync.dma_start(out=xtt[64:70, :], in_=xt[:, tB * F:(tB + 1) * F])

            # L1: [2]->[50] (padded 64); single K-stacked MM per tile
            p1 = ps1.tile([128, F], F32)
            nc.tensor.matmul(p1[0:64, :], wht[0:6, WH_WE1:WH_WE1 + 64],
                             xtt[0:6, :], start=True, stop=True,
                             tile_position=(0, 0))
            nc.tensor.matmul(p1[64:128, :], wht[64:70, WH_WE1:WH_WE1 + 64],
                             xtt[64:70, :], start=True, stop=True,
                             tile_position=(64, 64))
            h1_32 = sb.tile([128, F], F32, tag="h1_32")
            h1h = sb.tile([128, F], F16, tag="h1h")
            h1l = sb.tile([128, F], F16, tag="h1l")
            drain(p1, BB_BE1, h1_32, h1h, h1l, conv_on_act=True)

            # L2: [50]->[100] (padded 128); A rows 0-49, B rows 64-113
            p2a = ps2.tile([128, F], F32, tag="ps2")
            p2b = ps2.tile([128, F], F32, tag="ps2")
            mm3(p2a, 0, 128, 0, 50, WH_WE2, WL_WE2,
                h1h[0:50, :], h1l[0:50, :])
            mm3(p2b, 0, 128, 64, 50, WH_WE2, WL_WE2,
                h1h[64:114, :], h1l[64:114, :])
            hA32 = sb.tile([128, F], F32, tag="hA32")
            hAh = sb.tile([128, F], F16, tag="hAh")
            hAl = sb.tile([128, F], F16, tag="hAl")
            drain(p2a, BB_BE2, hA32, hAh, hAl)
            hB32 = sb.tile([128, F], F32, tag="hB32")
            hBh = sb.tile([128, F], F16, tag="hBh")
            hBl = sb.tile([128, F], F16, tag="hBl")
            drain(p2b, BB_BE2, hB32, hBh, hBl)

            # L3: [100]->[50] (padded 64); A at c0, B at c64
            p3 = ps3.tile([128, F], F32)
            mm3(p3, 0, 64, 0, 100, WH_WD1 + cA * 64, WL_WD1 + cA * 64,
                hAh[0:100, :], hAl[0:100, :])
            mm3(p3, 64, 64, 0, 100, WH_WD1 + cB * 64, WL_WD1 + cB * 64,
                hBh[0:100, :], hBl[0:100, :])
            a1_32 = sb.tile([128, F], F32, tag="a1_32")
            a1h = sb.tile([128, F], F16, tag="a1h")
            a1l = sb.tile([128, F], F16, tag="a1l")
            drain(p3, BB_BD1 + p, a1_32, a1h, a1l, conv_on_act=True)
            a1_pair[p % 2] = (a1h, a1l)

            if p % 2 == 1 or p == PAIRS - 1:
                # quad stage over tiles 4q .. 4q+3 (last quad: 2 tiles)
                q = p // 2
                full = p % 2 == 1
                a1ah, a1al = a1_pair[0]
                if full:
                    a1bh, a1bl = a1_pair[1]
                tq = 4 * q
                cs = [_tile_class(min(tq + i, T - 1)) for i in range(4)]

                # L4: [50]->[15] (padded 32); 4 tiles -> cols 0/32/64/96
                p4 = psl.tile([128, F], F32, tag="psl")
                mm3(p4, 0, 32, 0, 50, WH_WD2 + cs[0] * 32, WL_WD2 + cs[0] * 32,
                    a1ah[0:50, :], a1al[0:50, :])
                mm3(p4, 32, 32, 64, 50, WH_WD2 + cs[1] * 32, WL_WD2 + cs[1] * 32,
                    a1ah[64:114, :], a1al[64:114, :])
                if full:
                    mm3(p4, 64, 32, 0, 50, WH_WD2 + cs[2] * 32, WL_WD2 + cs[2] * 32,
                        a1bh[0:50, :], a1bl[0:50, :])
                    mm3(p4, 96, 32, 64, 50, WH_WD2 + cs[3] * 32, WL_WD2 + cs[3] * 32,
                        a1bh[64:114, :], a1bl[64:114, :])
                else:
                    nc.vector.memset(p4[64:128, :], 0.0)
                a2_32 = sbq.tile([128, F], F32, tag="a2_32")
                a2h = sbq.tile([128, F], F16, tag="a2h")
                a2l = sbq.tile([128, F], F16, tag="a2l")
                drain(p4, BB_BD2 + q, a2_32, a2h, a2l)

                # L5: [15]->[2] (padded 32); diag placement
                p5 = psl.tile([128, F], F32, tag="psl")
                mm3(p5, 0, 32, 0, 15, WH_WD3 + cs[0] * 32, WL_WD3 + cs[0] * 32,
                    a2h[0:15, :], a2l[0:15, :])
                mm3(p5, 32, 32, 32, 15, WH_WD3 + cs[1] * 32, WL_WD3 + cs[1] * 32,
                    a2h[32:47, :], a2l[32:47, :])
                if full:
                    mm3(p5, 64, 32, 64, 15, WH_WD3 + cs[2] * 32, WL_WD3 + cs[2] * 32,
                        a2h[64:79, :], a2l[64:79, :])
                    mm3(p5, 96, 32, 96, 15, WH_WD3 + cs[3] * 32, WL_WD3 + cs[3] * 32,
                        a2h[96:111, :], a2l[96:111, :])
                else:
                    nc.vector.memset(p5[64:128, :], 1.0)
                stage = sbq.tile([128, F], F32, tag="stage")
                nc.scalar.activation(
                    out=stage, in_=p5,
                    func=mybir.ActivationFunctionType.Copy, bias=0.0)

                # gather the 8 useful rows into the collector block
                b = q // 16
                r0 = (q % 16) * 8
                for i in range(4):
                    nc.sync.dma_start(
                        out=coll[r0 + 2 * i:r0 + 2 * i + 2, b * F:(b + 1) * F],
                        in_=stage[32 * i:32 * i + 2, :])

            if (p % 2 == 1 and (p // 2) % 16 == 15) or p == PAIRS - 1:
                # tail for completed block (values carry the SC scale, which
                # cancels in o/|o|; only the bias blob is pre-scaled)
                b = (p // 2) // 16
                colv = coll[:, b * F:(b + 1) * F]
                ob = sbq.tile([128, F], F32, tag="ob")
                nc.vector.tensor_scalar(ob, colv, tbt[:, b:b + 1], None, ADD)
                sq = sbq.tile([128, F], F32, tag="sq")
                nc.vector.tensor_tensor(sq, ob, ob, MULT)
                p6 = psl.tile([64, F], F32, tag="psl")
                nc.tensor.matmul(p6[0:64, :], w32t[0:128, W32_SSEL:W32_SSEL + 64],
                                 sq[0:128, :], start=True, stop=True)
                nrm = sbq.tile([64, F], F32, tag="nrm")
                nc.scalar.activation(out=nrm, in_=p6[0:64, :],
                                     func=mybir.ActivationFunctionType.Sqrt)
                rcp = sbq.tile([64, F], F32, tag="rcp")
                nc.vector.reciprocal(rcp, nrm)
                p7 = psl.tile([128, F], F32, tag="psl")
                nc.tensor.matmul(p7, w32t[0:64, W32_RREP:W32_RREP + 128],
                                 rcp[0:64, :], start=True, stop=True)
                ov = sbq.tile([128, F], F32, tag="ov")
                nc.vector.tensor_tensor(ov, ob, p7, MULT)
                oc = sbq.tile([128, F], F32, tag="oc")
                nc.vector.tensor_scalar(oc, ov, -EPS, EPS,
                                        mybir.AluOpType.max, mybir.AluOpType.min)
                nc.sync.dma_start(out=out[:, b * F:(b + 1) * F], in_=oc)

    nc.compile()
    return nc


def _split16(a):
    hi = a.astype(np.float16)
    lo = (a - hi.astype(np.float32)).astype(np.float16)
    return hi, lo


def _host_prep(x, y, We1, be1, We2, be2, Wd1, bd1, Wd2, bd2, Wd3, bd3):
    x = np.ascontiguousarray(np.asarray(x, dtype=np.float32))
    y = np.asarray(y).astype(np.int64)
    order = np.argsort(y, kind="stable")
    counts = np.bincount(y, minlength=C)

    # per-(class, core) contiguous shares; x scaled by SC and hi/lo split
    xs = x * np.float32(SC)
    xh = xs.astype(np.float16)
    xl = (xs - xh.astype(np.float32)).astype(np.float16)
    xt_cores = np.zeros((NCORES, 6, S), dtype=np.float16)
    seg_meta = []
    pos = 0
    for c in range(C):
        cnt = int(counts[c])
        base, rem = divmod(cnt, NCORES)
        off = 0
        for k in range(NCORES):
            n_k = base + (1 if k < rem else 0)
            assert n_k <= CAP, f"class {c} share {n_k} exceeds CAP {CAP}"
            idx = order[pos + off: pos + off + n_k]
            sl = slice(c * CAP, c * CAP + n_k)
            xt_cores[k, 0:2, sl] = xh[idx].T
            xt_cores[k, 2:4, sl] = xl[idx].T
            xt_cores[k, 4:6, sl] = xh[idx].T
            seg_meta.append((k, c, n_k, idx))
            off += n_k
        pos += cnt

    We1 = np.asarray(We1, np.float32); be1 = np.asarray(be1, np.float32)
    We2 = np.asarray(We2, np.float32); be2 = np.asarray(be2, np.float32)
    Wd1 = np.asarray(Wd1, np.float32); bd1 = np.asarray(bd1, np.float32)
    Wd2 = np.asarray(Wd2, np.float32); bd2 = np.asarray(bd2, np.float32)
    Wd3 = np.asarray(Wd3, np.float32); bd3 = np.asarray(bd3, np.float32)

    WH = np.zeros((128, WH_COLS), dtype=np.float16)
    e1h, e1l = _split16(We1)
    for r0 in (0, 64):
        WH[r0 + 0:r0 + 2, WH_WE1:WH_WE1 + 50] = e1h
        WH[r0 + 2:r0 + 4, WH_WE1:WH_WE1 + 50] = e1h
        WH[r0 + 4:r0 + 6, WH_WE1:WH_WE1 + 50] = e1l
    e2h, e2l = _split16(We2)
    for r0 in (0, 64):
        WH[r0:r0 + 50, WH_WE2:WH_WE2 + 100] = e2h
        WH[r0:r0 + 50, WL_WE2:WL_WE2 + 100] = e2l
    for c in range(C):
        d1h, d1l = _split16(Wd1[c])
        WH[0:100, WH_WD1 + c * 64:WH_WD1 + c * 64 + 50] = d1h
        WH[0:100, WL_WD1 + c * 64:WL_WD1 + c * 64 + 50] = d1l
        d2h, d2l = _split16(Wd2[c])
        for r0 in (0, 64):
            WH[r0:r0 + 50, WH_WD2 + c * 32:WH_WD2 + c * 32 + 15] = d2h
            WH[r0:r0 + 50, WL_WD2 + c * 32:WL_WD2 + c * 32 + 15] = d2l
        d3h, d3l = _split16(Wd3[c])
        for r0 in (0, 32, 64, 96):
            WH[r0:r0 + 15, WH_WD3 + c * 32:WH_WD3 + c * 32 + 2] = d3h
            WH[r0:r0 + 15, WL_WD3 + c * 32:WL_WD3 + c * 32 + 2] = d3l

    W32 = np.zeros((128, W32_COLS), dtype=np.float32)
    rr = np.arange(128)
    W32[rr, W32_SSEL + rr // 2] = 1.0 / (P_NORM_SCALE * P_NORM_SCALE)
    jj = np.arange(64)
    W32[jj, W32_RREP + 2 * jj] = 1.0
    W32[jj, W32_RREP + 2 * jj + 1] = 1.0

    BB = np.zeros((128, BB_COLS), dtype=np.float32)
    BB[0:50, BB_BE1] = be1 * SC
    BB[64:114, BB_BE1] = be1 * SC
    BB[0:100, BB_BE2] = be2 * SC
    for p in range(PAIRS):
        cA, cB = _tile_class(2 * p), _tile_class(2 * p + 1)
        BB[0:50, BB_BD1 + p] = bd1[cA] * SC
        BB[64:114, BB_BD1 + p] = bd1[cB] * SC
    for q in range(QUADS):
        cs = [_tile_class(min(4 * q + i, T - 1)) for i in range(4)]
        BB[0:15, BB_BD2 + q] = bd2[cs[0]] * SC
        BB[32:47, BB_BD2 + q] = bd2[cs[1]] * SC
        BB[64:79, BB_BD2 + q] = bd2[cs[2]] * SC
        BB[96:111, BB_BD2 + q] = bd2[cs[3]] * SC

    TB = np.zeros((128, BLOCKS), dtype=np.float32)
    for t in range(T):
        q = t // 4
        b, row0 = q // 16, (q % 16) * 8 + (t % 4) * 2
        cls = _tile_class(t)
        TB[row0, b] = bd3[cls, 0] * SC
        TB[row0 + 1, b] = bd3[cls, 1] * SC

    return xt_cores, WH, W32, BB, TB, seg_meta


def _unpack(results, seg_meta):
    final = np.empty((N, 2), dtype=np.float32)
    per_core = []
    for k in range(NCORES):
        OUT = results[k]["out"]
        o = np.empty((S, 2), dtype=np.float32)
        for t in range(T):
            q, r = t // 4, t % 4
            b, row0 = q // 16, (q % 16) * 8 + r * 2
            o[t * F:(t + 1) * F, 0] = OUT[row0, b * F:(b + 1) * F]
            o[t * F:(t + 1) * F, 1] = OUT[row0 + 1, b * F:(b + 1) * F]
        per_core.append(o)
    for (k, c, n_k, idx) in seg_meta:
        final[idx] = per_core[k][c * CAP:c * CAP + n_k]
    return final


def kernel(**inputs):
    if "nc" not in _CACHE:
        _CACHE["nc"] = build_program()
    nc = _CACHE["nc"]

    xt_cores, WH, W32, BB, TB, seg_meta = _host_prep(**inputs)

    if os.environ.get("BASS_KERNEL_SIM"):
        from concourse.bass_interp import CoreSim
        results = []
        for k in range(int(os.environ.get("BASS_KERNEL_SIM_CORES", "1"))):
            sim = CoreSim(nc, trace=False, require_finite=False,
                          require_nnan=False)
            sim.tensor("xt")[:] = xt_cores[k]
            sim.tensor("wh")[:] = WH
            sim.tensor("w32")[:] = W32
            sim.tensor("bb")[:] = BB
            sim.tensor("tb")[:] = TB
            sim.simulate(check_with_hw=False)
            results.append({"out": np.array(sim.tensor("out"))})
        while len(results) < NCORES:
            results.append(results[-1])
    else:
        from concourse.bass_utils import run_bass_kernel_spmd
        in_maps = [{"xt": xt_cores[k], "wh": WH, "w32": W32, "bb": BB,
                    "tb": TB} for k in range(NCORES)]
        res = run_bass_kernel_spmd(nc, in_maps, core_ids=list(range(NCORES)),
                                   **_CACHE.get("run_kwargs", {}))
        _CACHE["last_result"] = res
        results = res.results

    return _unpack(results, seg_meta)
